# revision 65
# baseline (speedup 1.0000x reference)
"""Trainium2 Bass kernel for nn_CrossAttention_51539607552970.

Sharding: 8 cores = 2 (batch) x 4 (GQA kv-head groups). Each core computes
4 query heads + its single kv head for one batch element, producing a
partial output (its head-group's contribution through wo); the host sums
the 4 partials per batch element (tensor-parallel unshard).

v4 (from v3 @ 377us):
- All-fp16 data path: x/c/wq/wkv/wo cast to fp16 on host, DMA'd fp16
  (21MB vs 41MB in), output staged+DMA'd fp16 and upcast on host.
- Two HWDGE queues: c-stream on SP, x-stream + weights on Activation.
- PSUM re-plan: scores double-buffer (2x[128,1024]) + attention
  accumulators (2x[128,512]) + work rotation (2x[128,512]) so wo/qproj
  never recycle the score banks.
- Fine-grained fill: wo(prev block) and qproj(future block) matmuls are
  pumped one unit per attention iteration, keeping the PE continuously
  busy (max p-state).
- Scalar keeps only exp/square/ln-exp-rsqrt (one activation table, no
  ACT_TABLE churn); output copies moved to the Pool engine; rmsnorm
  1/sqrt via Ln+Exp on Scalar instead of Sqrt+DVE-reciprocal.
"""

import sys

sys.path.insert(0, "/opt/trn_rl_repo")

import numpy as np

import concourse.bass as bass
import concourse.mybir as mybir
import concourse.tile as tile
from concourse import bacc
from concourse.bass_utils import run_bass_kernel_spmd
from concourse.masks import make_identity

F32 = mybir.dt.float32
F16 = mybir.dt.float16
AF = mybir.ActivationFunctionType
OP = mybir.AluOpType

# Problem constants (hardcoded per contract).
B, S, L = 2, 2048, 2048
H, KVH, D = 16, 4, 128
HID = H * D
EPS = 1e-6
SCALE = 1.0 / np.sqrt(D)

NH = 4           # query heads per core
P = 128          # partitions
HC = HID // P    # 16 hid chunks
KC = L // P      # 16 key chunks
NB = 4           # 512-wide blocks per 2048 (both keys and queries)
PIPE = 3         # P@V lags score/exp by this many key-chunks

_compiled = None


def _build():
    nc = bacc.Bacc("TRN2", num_devices=8)

    xT = nc.dram_tensor("xT", [P, HC, S], F16, kind="ExternalInput")
    cT = nc.dram_tensor("cT", [P, HC, L], F16, kind="ExternalInput")
    wq = nc.dram_tensor("wq", [P, HC, NH * D], F16, kind="ExternalInput")
    wkv = nc.dram_tensor("wkv", [P, HC, 2 * D], F16, kind="ExternalInput")
    wo = nc.dram_tensor("wo", [P, NH, HID], F16, kind="ExternalInput")
    nqw = nc.dram_tensor("nqw", [P, 1], F32, kind="ExternalInput")
    nkw = nc.dram_tensor("nkw", [P, 1], F32, kind="ExternalInput")
    out = nc.dram_tensor("out", [S, HID], F16, kind="ExternalOutput")

    with nc.allow_low_precision(reason="fp16 matmul input rounding"), \
         tile.TileContext(nc) as tc:
        with tc.tile_pool(name="consts", bufs=1) as consts, \
             tc.tile_pool(name="weights", bufs=1) as weights, \
             tc.tile_pool(name="cstream", bufs=12) as cstream, \
             tc.tile_pool(name="xstream", bufs=12) as xstream, \
             tc.tile_pool(name="kv", bufs=1) as kvpool, \
             tc.tile_pool(name="xqt", bufs=1) as xqtpool, \
             tc.tile_pool(name="small", bufs=1) as small, \
             tc.tile_pool(name="esbp", bufs=5) as esbp, \
             tc.tile_pool(name="accp", bufs=2) as accp, \
             tc.tile_pool(name="attnp", bufs=12) as attnp, \
             tc.tile_pool(name="otap", bufs=16) as otap, \
             tc.tile_pool(name="outp", bufs=2) as outp, \
             tc.tile_pool(name="psum", bufs=1, space="PSUM") as psum:

            # ---- constants ----
            ones_f = consts.tile([P, P], F32)
            nc.vector.memset(ones_f[:], 1.0)
            ones_h = consts.tile([P, P], F16)
            nc.scalar.copy(ones_h[:], ones_f[:])
            ident_f = consts.tile([P, P], F32)
            make_identity(nc, ident_f)
            ident_h = consts.tile([P, P], F16)
            nc.scalar.copy(ident_h[:], ident_f[:])
            nqw_sb = consts.tile([P, 1], F32)
            nc.sync.dma_start(nqw_sb[:], nqw[:])
            nkw_sb = consts.tile([P, 1], F32)
            nc.sync.dma_start(nkw_sb[:], nkw[:])
            eps_sb = consts.tile([P, 1], F32)
            nc.vector.memset(eps_sb[:], EPS)
            # Pin the Scalar activation table to natural_log_exp_and_others
            # (exp + ln + copy + square) so the auto-inserter never churns.
            nc.scalar.add_instruction(mybir.InstLoadActFuncSet(
                name=nc.get_next_instruction_name(), act_func_set_id=6,
                ins=[], outs=[]))

            # ---- resident weights ----
            wq_sb = weights.tile([P, HC * NH * D], F16)
            wkv_sbs = [weights.tile([P, 4 * 2 * D], F16, name=f"wkv{i}")
                       for i in range(4)]
            wo_sb = weights.tile([P, NH * HID], F16)

            # ---- persistent activations ----
            kT_sb = kvpool.tile([P, L], F16)              # [D, keys]
            v_sb = kvpool.tile([P, KC * D], F16)          # kt-th block = [keys, D]
            xqT_list = [xqtpool.tile([P, S], F16, name=f"xqT{h}")
                        for h in range(NH)]

            def dma_stream(eng, pool, src, blk):
                tiles = []
                for hp in range(HC // 2):
                    t = pool.tile([P, 2 * 512], F16, name="st", tag="stream")
                    eng.dma_start(
                        t[:], src[:, 2 * hp:2 * hp + 2,
                                  blk * 512:(blk + 1) * 512])
                    tiles.append(t)
                return tiles

            def rsqrt_chain(sum_ps, width, tagb):
                """1/sqrt(sum/D + eps) via Ln+Exp (stays in the exp table)."""
                ln_t = small.tile([P, width], F32, name="ln", tag=f"ln{tagb}")
                nc.scalar.activation(ln_t[:], sum_ps[:, 0:width], AF.Ln,
                                     bias=eps_sb[:], scale=1.0 / D)
                rr = small.tile([P, width], F32, name="rr", tag=f"rsq{tagb}")
                nc.scalar.activation(rr[:], ln_t[:], AF.Exp, scale=-0.5)
                return rr

            def make_kv_units(kcol, ct_tiles):
                """K/V projection + k-rmsnorm + V transpose for one key col,
                as 4 fill units (3 matmul batches + finish)."""
                state = {}

                def mms(lo, hi):
                    def f():
                        if lo == 0:
                            state["k"] = psum.tile([P, 512], F32, name="k_ps",
                                                   tag="work", bufs=2)
                            state["v"] = psum.tile([P, 512], F32, name="v_ps",
                                                   tag="work", bufs=2)
                        k_ps, v_ps = state["k"], state["v"]
                        for hc in range(lo, hi):
                            src = ct_tiles[hc // 2][
                                :, (hc % 2) * 512:(hc % 2 + 1) * 512]
                            wv = wkv_sbs[hc // 4]
                            o = (hc % 4) * 256
                            nc.tensor.matmul(
                                k_ps[:], wv[:, o:o + 128],
                                src, start=(hc == 0), stop=(hc == HC - 1))
                            nc.tensor.matmul(
                                v_ps[:], wv[:, o + 128:o + 256],
                                src, start=(hc == 0), stop=(hc == HC - 1))
                    return f

                def finish():
                    k_ps, v_ps = state["k"], state["v"]
                    # copy k/v out of PSUM at once so the work slots free
                    # fast; the rmsnorm chain runs SBUF-side.
                    ktmp = small.tile([P, 512], F16, name="ktmp", tag="ktmp")
                    nc.vector.tensor_copy(ktmp[:], k_ps[:])
                    vT = small.tile([P, 512], F16, name="vT", tag="vT")
                    nc.vector.tensor_copy(vT[:], v_ps[:])
                    # transpose v 128x128 blocks into [keys, D] FIRST: their
                    # PSUM slots recycle via fast DVE copies, keeping the PE
                    # clear of the Scalar-paced rmsnorm chain below.
                    for j in range(4):
                        kt = kcol * 4 + j
                        tp = psum.tile([P, P], F16, name="tp", tag="work",
                                       bufs=2)
                        nc.tensor.transpose(tp[:], vT[:, j * P:(j + 1) * P],
                                            ident_h[:])
                        nc.vector.tensor_copy(v_sb[:, kt * D:(kt + 1) * D],
                                              tp[:])
                    # k rmsnorm over D (partition dim): sumsq via ones matmul
                    ksq = small.tile([P, 512], F16, name="ksq", tag="sq")
                    nc.scalar.square(ksq[:], ktmp[:])
                    ksum = psum.tile([P, 512], F32, name="ksum", tag="work",
                                     bufs=2)
                    nc.tensor.matmul(ksum[:], ones_h[:], ksq[:],
                                     start=True, stop=True)
                    krr = rsqrt_chain(ksum, 512, "k")
                    nc.vector.scalar_tensor_tensor(
                        out=kT_sb[:, kcol * 512:(kcol + 1) * 512],
                        in0=ktmp[:], scalar=nkw_sb[:], in1=krr[:],
                        op0=OP.mult, op1=OP.mult)

                return [("kv", mms(0, 6)), ("kv", mms(6, 12)),
                        ("kv", mms(12, 16)), ("kv", finish)]

            def emit_kv(kcol, ct_tiles):
                for _, fn in make_kv_units(kcol, ct_tiles):
                    fn()

            def qp_head_mms(pb, h, xt_tiles):
                """Q projection matmuls for one head of one 512-query block.

                The PSUM result is copied straight to SBUF (one quick DVE op)
                so the work-psum slot frees immediately; the rmsnorm chain
                then runs SBUF-side without blocking the PE rotation.
                """
                q_ps = psum.tile([P, 512], F32, name=f"q_ps{h}", tag="work",
                                 bufs=2)
                for hc in range(HC):
                    src = xt_tiles[hc // 2][:, (hc % 2) * 512:(hc % 2 + 1) * 512]
                    nc.tensor.matmul(
                        q_ps[:],
                        wq_sb[:, hc * 512 + h * D:hc * 512 + (h + 1) * D],
                        src, start=(hc == 0), stop=(hc == HC - 1))
                qtmp = small.tile([P, 512], F16, name="qtmp",
                                  tag=f"qtmp{h % 2}")
                nc.vector.tensor_copy(qtmp[:], q_ps[:])
                qsq = small.tile([P, 512], F16, name="qsq", tag=f"sq{h % 2}")
                nc.vector.tensor_tensor(out=qsq[:], in0=qtmp[:], in1=qtmp[:],
                                        op=OP.mult)
                return qtmp, qsq

            def qp_head_norm(pb, h, qtmp, qsq):
                """RMSNorm for one head's q block (after mms + square)."""
                qsum = psum.tile([P, 512], F32, name=f"qsum{h}", tag="work",
                                 bufs=2)
                nc.tensor.matmul(qsum[:], ones_h[:], qsq[:],
                                 start=True, stop=True)
                qrr = rsqrt_chain(qsum, 512, f"q{h % 2}")
                nc.vector.scalar_tensor_tensor(
                    out=xqT_list[h][:, pb * 512:(pb + 1) * 512],
                    in0=qtmp[:], scalar=nqw_sb[:], in1=qrr[:],
                    op0=OP.mult, op1=OP.mult)

            def make_qp_fills(pb, xt_tiles):
                """Fill units (kind, closure) for one q-block's projection."""
                units = []
                state = {}

                def mk_a(h):
                    def f():
                        state[h] = qp_head_mms(pb, h, xt_tiles)
                    return f

                def mk_b(h):
                    def f():
                        q_ps, qsq = state.pop(h)
                        qp_head_norm(pb, h, q_ps, qsq)
                    return f

                for h in range(NH):
                    units.append(("qpa", mk_a(h)))
                    units.append(("qpb", mk_b(h)))
                return units

            def make_wo_fills(ab, attn_map):
                """Fill units for one block's output projection (16 chunks)."""
                units = []
                state = {}

                def mk(qs, ht):
                    def f():
                        if ht == 0:
                            state["ot"] = outp.tile([P, 2048], F16, name="ot",
                                                    tag="ot")
                        w_ps = psum.tile([P, 512], F32, name="w_ps",
                                         tag="work", bufs=2)
                        for h in range(NH):
                            nc.tensor.matmul(
                                w_ps[:],
                                attn_map[h][:, qs * P:(qs + 1) * P],
                                wo_sb[:, h * HID + ht * 512:
                                      h * HID + (ht + 1) * 512],
                                start=(h == 0), stop=(h == NH - 1))
                        ot = state["ot"]
                        nc.vector.tensor_copy(
                            ot[:, ht * 512:(ht + 1) * 512], w_ps[:])
                        if ht == 3:
                            q0 = ab * 512 + qs * P
                            nc.sync.dma_start(out[q0:q0 + P, :], ot[:])
                    return f

                ready = lambda: 3 in attn_map  # noqa: E731
                for qs in range(4):
                    for ht in range(4):
                        units.append(("wo", mk(qs, ht), ready))
                return units

            def make_wo_a_fills(ab, attn_map, ota_store):
                """First-pass wo chunks (heads 0-1 only) for the LAST block;
                pumped during its stream once hg0's post has run. Partials
                land in fp16 SBUF for the tail pass to combine."""
                units = []
                ready = lambda: 0 in attn_map  # noqa: E731

                def mk(qs, ht):
                    def f():
                        w_ps = psum.tile([P, 512], F32, name="w_ps",
                                         tag="work", bufs=2)
                        for h in range(2):
                            nc.tensor.matmul(
                                w_ps[:],
                                attn_map[h][:, qs * P:(qs + 1) * P],
                                wo_sb[:, h * HID + ht * 512:
                                      h * HID + (ht + 1) * 512],
                                start=(h == 0), stop=(h == 1))
                        ota = otap.tile([P, 512], F16, name="ota", tag="ota",
                                        bufs=16)
                        nc.vector.tensor_copy(ota[:], w_ps[:])
                        ota_store[(qs, ht)] = ota
                    return f

                for qs in range(4):
                    for ht in range(4):
                        units.append(("wo", mk(qs, ht), ready))
                return units

            def emit_wo_b(ab, attn_map, ota_store):
                """Tail pass: heads 2-3 + combine with the stored partials."""
                for qs in range(4):
                    ot = outp.tile([P, 2048], F16, name="ot", tag="ot")
                    for ht in range(4):
                        w_ps = psum.tile([P, 512], F32, name="w_ps",
                                         tag="work", bufs=2)
                        for h in (2, 3):
                            nc.tensor.matmul(
                                w_ps[:],
                                attn_map[h][:, qs * P:(qs + 1) * P],
                                wo_sb[:, h * HID + ht * 512:
                                      h * HID + (ht + 1) * 512],
                                start=(h == 2), stop=(h == 3))
                        nc.vector.tensor_tensor(
                            out=ot[:, ht * 512:(ht + 1) * 512], in0=w_ps[:],
                            in1=ota_store[(qs, ht)][:], op=OP.add)
                    q0 = ab * 512 + qs * P
                    nc.sync.dma_start(out[q0:q0 + P, :], ot[:])

            class Pump:
                def __init__(self):
                    self.q = []
                    self.cooldown = 0

                def add(self, units):
                    self.q.extend(units)

                def step(self):
                    if self.cooldown > 0:
                        self.cooldown -= 1
                        return
                    if not self.q:
                        return
                    idx = None
                    for i, unit in enumerate(self.q):
                        if len(unit) <= 2 or unit[2]():
                            idx = i
                            break
                    if idx is None:
                        return  # everything gated this tick
                    unit = self.q.pop(idx)
                    kind, fn = unit[0], unit[1]
                    fn()
                    # qpa->qpb needs a few iters for the DVE copy+square to
                    # land before the qsum matmul; wo units are cheap and can
                    # fire every iteration.
                    self.cooldown = {"qpa": 2, "qpb": 0, "wo": 0,
                                     "kv": 0}[kind]

                def drain(self):
                    while self.q:
                        unit = self.q.pop(0)
                        unit[1]()

            pump = Pump()

            def make_post(hg, att0, att1, tree, attn_map):
                """Deferred sump + normalize for one head group; emitted a
                couple of iterations into the NEXT stream so the DVE add
                tree and PSUM frees never stall the PE."""
                def post():
                    acc = tree[3]
                    sump0 = psum.tile([P, 512], F32, name="sump0",
                                      tag="work", bufs=2)
                    nc.tensor.matmul(sump0[:], ones_h[:], acc[:, 0:512],
                                     start=True, stop=True)
                    sump1 = psum.tile([P, 512], F32, name="sump1",
                                      tag="work", bufs=2)
                    nc.tensor.matmul(sump1[:], ones_h[:], acc[:, 512:1024],
                                     start=True, stop=True)
                    for hh, (att, sump) in ((0, (att0, sump0)),
                                            (1, (att1, sump1))):
                        h = 2 * hg + hh
                        rr = small.tile([P, 512], F32, name="rr",
                                        tag=f"den{hh}")
                        nc.vector.reciprocal_approx_fast(out=rr[:],
                                                         in_=sump[:])
                        attn = attnp.tile([P, 512], F16, name=f"attn{h}",
                                          tag="attn")
                        nc.vector.tensor_tensor(
                            out=attn[:], in0=att[:], in1=rr[:], op=OP.mult)
                        attn_map[h] = attn
                return post

            def run_attention(block_fills, attn_maps):
                """All 4 blocks x 2 head groups as ONE software-pipelined
                stream: scores/exp for stream si overlap the P@V tail of
                stream si-1, so the PE and Scalar never re-prime at
                boundaries. Posts (sump+normalize) are deferred into the
                following stream; fill units pump every tick.

                Returns the final deferred post (for the tail).
                """
                NS = 2 * NB
                state = {}
                pending = None
                for t in range(NS * KC + PIPE):
                    if t < NS * KC:
                        si, kt = divmod(t, KC)
                        ab, hg = divmod(si, 2)
                        if kt == 0:
                            if hg == 0:
                                pump.add(block_fills[ab])
                            state[si] = {
                                "es": [None] * KC,
                                "tree": [None] * 5,
                                "att0": psum.tile([P, 512], F32,
                                                  name=f"att{2 * hg}",
                                                  tag="att", bufs=2),
                                "att1": psum.tile([P, 512], F32,
                                                  name=f"att{2 * hg + 1}",
                                                  tag="att", bufs=2),
                            }
                        # the previous stream's trailing AVs are emitted up
                        # to kt==PIPE-1 of this stream; its post must follow
                        # them or the normalize reads a partial accumulator
                        if kt == PIPE and pending is not None:
                            pending()
                            pending = None
                        st = state[si]
                        h0, h1 = 2 * hg, 2 * hg + 1
                        q0 = ab * 512
                        stp = psum.tile([P, 1024], F32, name="stp",
                                        tag="stp", bufs=2)
                        nc.tensor.matmul(stp[:, 0:512],
                                         kT_sb[:, kt * P:(kt + 1) * P],
                                         xqT_list[h0][:, q0:q0 + 512],
                                         start=True, stop=True)
                        nc.tensor.matmul(stp[:, 512:1024],
                                         kT_sb[:, kt * P:(kt + 1) * P],
                                         xqT_list[h1][:, q0:q0 + 512],
                                         start=True, stop=True)
                        es = esbp.tile([P, 1024], F16, name="es", tag="es")
                        nc.scalar.activation(es[:], stp[:], AF.Exp)
                        st["es"][kt] = es
                        # pairwise-tree denominator accumulation on DVE
                        if kt % 2 == 1:
                            tree = st["tree"]
                            t0 = accp.tile([P, 1024], F16, name="acc0",
                                           tag="acc0", bufs=3)
                            nc.vector.tensor_tensor(
                                out=t0[:], in0=st["es"][kt - 1][:],
                                in1=st["es"][kt][:], op=OP.add)
                            lv = 0
                            while tree[lv] is not None:
                                t1 = accp.tile([P, 1024], F16,
                                               name=f"acc{lv + 1}",
                                               tag=f"acc{lv + 1}", bufs=2)
                                nc.vector.tensor_tensor(
                                    out=t1[:], in0=tree[lv][:], in1=t0[:],
                                    op=OP.add)
                                tree[lv] = None
                                t0 = t1
                                lv += 1
                            tree[lv] = t0
                        if kt == KC - 1:
                            pending = make_post(hg, st["att0"], st["att1"],
                                                st["tree"], attn_maps[ab])
                    # P@V leg, lagging PIPE ticks
                    tj = t - PIPE
                    if tj >= 0:
                        sj, ktj = divmod(tj, KC)
                        stj = state[sj]
                        pes = stj["es"][ktj]
                        nc.tensor.matmul(
                            stj["att0"][:], v_sb[:, ktj * D:(ktj + 1) * D],
                            pes[:, 0:512],
                            start=(ktj == 0), stop=(ktj == KC - 1))
                        nc.tensor.matmul(
                            stj["att1"][:], v_sb[:, ktj * D:(ktj + 1) * D],
                            pes[:, 512:1024],
                            start=(ktj == 0), stop=(ktj == KC - 1))
                        if ktj == KC - 1:
                            del state[sj]
                    pump.step()
                return pending

            # ======== schedule ========
            # SP wire:  wkv+ct0 interleaved, ct1, ct2, ct3 (+ out stores).
            # ACT wire: wq, xt0, xt1, xt2, xt3, wo — all land during the
            #           startup phase while the Scalar sequencer is idle.
            # wkv split across both queues; c on SP, x/weights on ACT so the
            # two input streams land in parallel.
            for i in range(4):
                eng = nc.sync if i % 2 == 0 else nc.scalar
                eng.dma_start(wkv_sbs[i][:], wkv[:, 4 * i:4 * (i + 1), :])
            ct0 = []
            for hp in range(8):
                t = cstream.tile([P, 2 * 512], F16, name="st", tag="stream")
                nc.sync.dma_start(t[:], cT[:, 2 * hp:2 * hp + 2, 0:512])
                ct0.append(t)
            xt0 = dma_stream(nc.scalar, xstream, xT, 0)
            nc.scalar.dma_start(wq_sb[:], wq[:, :, :])
            ct1 = dma_stream(nc.sync, cstream, cT, 1)

            # startup: KV cols 0-2 + q-projection of heads 0/1 (block 0's
            # first head group); everything else rides the fill queue.
            qp0 = make_qp_fills(0, xt0)  # [A0 B0 A1 B1 A2 B2 A3 B3]
            emit_kv(0, ct0)
            qp0[0][1]()   # A(h0)
            qp0[2][1]()   # A(h1)
            ct2 = dma_stream(nc.sync, cstream, cT, 2)
            emit_kv(1, ct1)
            xt1 = dma_stream(nc.scalar, xstream, xT, 1)
            qp0[1][1]()   # B(h0)
            qp0[3][1]()   # B(h1)
            ct3 = dma_stream(nc.sync, cstream, cT, 3)
            emit_kv(2, ct2)
            xt2 = dma_stream(nc.scalar, xstream, xT, 2)
            xt3 = dma_stream(nc.scalar, xstream, xT, 3)
            nc.scalar.dma_start(wo_sb[:], wo[:, :, :])

            # one pipelined attention stream over all 4 blocks; KV col 3,
            # QP0's second head group, later q-projections and all output
            # projections ride the fill queue
            attn_maps = [{} for _ in range(NB)]
            ota_store = {}
            block_fills = {
                0: (make_kv_units(3, ct3) + qp0[4:]
                    + make_qp_fills(1, xt1) + make_qp_fills(2, xt2)),
                1: make_qp_fills(3, xt3) + make_wo_fills(0, attn_maps[0]),
                2: make_wo_fills(1, attn_maps[1]),
                # wo_a first: its guard holds it until block 3's hg0 post,
                # and the skip-scan pulls WO2 units through as fillers
                3: (make_wo_a_fills(3, attn_maps[3], ota_store)
                    + make_wo_fills(2, attn_maps[2])),
            }
            pending = run_attention(block_fills, attn_maps)
            # leftover wo_a units cover the DVE add-tree latency before the
            # final post's sump matmuls need it
            pump.drain()
            pending()
            emit_wo_b(3, attn_maps[3], ota_store)

    nc.compile()
    return nc


def _get_compiled():
    global _compiled
    if _compiled is None:
        _compiled = _build()
    return _compiled


def _to3d(a):
    """[rows=HC*P, cols] -> [P, HC, cols] (partition-major chunks)."""
    cols = a.shape[1]
    return np.ascontiguousarray(
        a.reshape(HC, P, cols).transpose(1, 0, 2))


def _shard_inputs(x, c, wq, wkv, wo, norm_q_w, norm_k_w):
    x = np.asarray(x, np.float32)
    c = np.asarray(c, np.float32)
    wq = np.asarray(wq, np.float32)
    wkv = np.asarray(wkv, np.float32)
    wo = np.asarray(wo, np.float32)
    nqw = (np.asarray(norm_q_w, np.float32) * np.float32(SCALE)).reshape(P, 1)
    nkw = np.asarray(norm_k_w, np.float32).reshape(P, 1).copy()

    xTs = [_to3d(np.ascontiguousarray(x[b].T)).astype(np.float16)
           for b in range(B)]
    cTs = [_to3d(np.ascontiguousarray(c[b].T)).astype(np.float16)
           for b in range(B)]
    in_maps = []
    for core in range(8):
        b, g = core // 4, core % 4
        blk = wkv[:, g * 256:(g + 1) * 256]
        kvpack = np.concatenate([blk[:, 0::2], blk[:, 1::2]], axis=1)
        wo_g = wo[g * 512:(g + 1) * 512, :]
        in_maps.append({
            "xT": xTs[b],
            "cT": cTs[b],
            "wq": _to3d(wq[:, g * 512:(g + 1) * 512]).astype(np.float16),
            "wkv": _to3d(kvpack).astype(np.float16),
            "wo": np.ascontiguousarray(
                wo_g.reshape(NH, P, HID).transpose(1, 0, 2)).astype(
                    np.float16),
            "nqw": nqw,
            "nkw": nkw,
        })
    return in_maps


def run_sharded(inputs, trace=False, trace_cores=None):
    """Run the SPMD kernel; returns (full_output, BassKernelResults)."""
    nc = _get_compiled()
    in_maps = _shard_inputs(**inputs)
    res = run_bass_kernel_spmd(nc, in_maps, core_ids=list(range(8)),
                               trace=trace, trace_cores=trace_cores)
    parts = [r["out"] for r in res.results]
    full = np.empty((B, S, HID), np.float32)
    for b in range(B):
        full[b] = np.sum(np.stack([parts[4 * b + g] for g in range(4)], 0),
                         axis=0, dtype=np.float64).astype(np.float32)
    return full, res


def kernel(**inputs) -> np.ndarray:
    out, _ = run_sharded(inputs, trace=False)
    return out


# revision 68
# speedup vs baseline: 1.0248x; 1.0248x over previous
"""Trainium2 Bass kernel for nn_CrossAttention_51539607552970.

Sharding: 8 cores = 2 (batch) x 4 (GQA kv-head groups). Each core computes
4 query heads + its single kv head for one batch element, producing a
partial output (its head-group's contribution through wo); the host sums
the 4 partials per batch element (tensor-parallel unshard).

v4 (from v3 @ 377us):
- All-fp16 data path: x/c/wq/wkv/wo cast to fp16 on host, DMA'd fp16
  (21MB vs 41MB in), output staged+DMA'd fp16 and upcast on host.
- Two HWDGE queues: c-stream on SP, x-stream + weights on Activation.
- PSUM re-plan: scores double-buffer (2x[128,1024]) + attention
  accumulators (2x[128,512]) + work rotation (2x[128,512]) so wo/qproj
  never recycle the score banks.
- Fine-grained fill: wo(prev block) and qproj(future block) matmuls are
  pumped one unit per attention iteration, keeping the PE continuously
  busy (max p-state).
- Scalar keeps only exp/square/ln-exp-rsqrt (one activation table, no
  ACT_TABLE churn); output copies moved to the Pool engine; rmsnorm
  1/sqrt via Ln+Exp on Scalar instead of Sqrt+DVE-reciprocal.
"""

import sys

sys.path.insert(0, "/opt/trn_rl_repo")

import numpy as np

import concourse.bass as bass
import concourse.mybir as mybir
import concourse.tile as tile
from concourse import bacc
from concourse.bass_utils import run_bass_kernel_spmd
from concourse.masks import make_identity

F32 = mybir.dt.float32
F16 = mybir.dt.float16
AF = mybir.ActivationFunctionType
OP = mybir.AluOpType

# Problem constants (hardcoded per contract).
B, S, L = 2, 2048, 2048
H, KVH, D = 16, 4, 128
HID = H * D
EPS = 1e-6
SCALE = 1.0 / np.sqrt(D)

NH = 4           # query heads per core
P = 128          # partitions
HC = HID // P    # 16 hid chunks
KC = L // P      # 16 key chunks
NB = 4           # 512-wide blocks per 2048 (both keys and queries)
PIPE = 3         # P@V lags score/exp by this many key-chunks

_compiled = None


def _build():
    nc = bacc.Bacc("TRN2", num_devices=8)

    xT = nc.dram_tensor("xT", [P, HC, S], F16, kind="ExternalInput")
    cT = nc.dram_tensor("cT", [P, HC, L], F16, kind="ExternalInput")
    wq = nc.dram_tensor("wq", [P, HC, NH * D], F16, kind="ExternalInput")
    wkv = nc.dram_tensor("wkv", [P, HC, 2 * D], F16, kind="ExternalInput")
    wo = nc.dram_tensor("wo", [P, NH, HID], F16, kind="ExternalInput")
    nqw = nc.dram_tensor("nqw", [P, 1], F32, kind="ExternalInput")
    nkw = nc.dram_tensor("nkw", [P, 1], F32, kind="ExternalInput")
    out = nc.dram_tensor("out", [S, HID], F16, kind="ExternalOutput")

    with nc.allow_low_precision(reason="fp16 matmul input rounding"), \
         tile.TileContext(nc) as tc:
        with tc.tile_pool(name="consts", bufs=1) as consts, \
             tc.tile_pool(name="weights", bufs=1) as weights, \
             tc.tile_pool(name="cstream", bufs=12) as cstream, \
             tc.tile_pool(name="xstream", bufs=12) as xstream, \
             tc.tile_pool(name="kv", bufs=1) as kvpool, \
             tc.tile_pool(name="xqt", bufs=1) as xqtpool, \
             tc.tile_pool(name="small", bufs=1) as small, \
             tc.tile_pool(name="esbp", bufs=5) as esbp, \
             tc.tile_pool(name="accp", bufs=2) as accp, \
             tc.tile_pool(name="attnp", bufs=12) as attnp, \
             tc.tile_pool(name="otap", bufs=16) as otap, \
             tc.tile_pool(name="outp", bufs=2) as outp, \
             tc.tile_pool(name="psum", bufs=1, space="PSUM") as psum:

            # ---- constants ----
            ones_f = consts.tile([P, P], F32)
            nc.vector.memset(ones_f[:], 1.0)
            ones_h = consts.tile([P, P], F16)
            nc.scalar.copy(ones_h[:], ones_f[:])
            ident_f = consts.tile([P, P], F32)
            make_identity(nc, ident_f)
            ident_h = consts.tile([P, P], F16)
            nc.scalar.copy(ident_h[:], ident_f[:])
            nqw_sb = consts.tile([P, 1], F32)
            nc.sync.dma_start(nqw_sb[:], nqw[:])
            nkw_sb = consts.tile([P, 1], F32)
            nc.sync.dma_start(nkw_sb[:], nkw[:])
            eps_sb = consts.tile([P, 1], F32)
            nc.vector.memset(eps_sb[:], EPS)
            # Pin the Scalar activation table to natural_log_exp_and_others
            # (exp + ln + copy + square) so the auto-inserter never churns.
            nc.scalar.add_instruction(mybir.InstLoadActFuncSet(
                name=nc.get_next_instruction_name(), act_func_set_id=6,
                ins=[], outs=[]))

            # ---- resident weights ----
            wq_sb = weights.tile([P, HC * NH * D], F16)
            wkv_sbs = [weights.tile([P, 4 * 2 * D], F16, name=f"wkv{i}")
                       for i in range(4)]
            wo_sb = weights.tile([P, NH * HID], F16)

            # ---- persistent activations ----
            kT_sb = kvpool.tile([P, L], F16)              # [D, keys]
            v_sb = kvpool.tile([P, KC * D], F16)          # kt-th block = [keys, D]
            xqT_list = [xqtpool.tile([P, S], F16, name=f"xqT{h}")
                        for h in range(NH)]

            def dma_stream(eng, pool, src, blk):
                tiles = []
                for hp in range(HC // 2):
                    t = pool.tile([P, 2 * 512], F16, name="st", tag="stream")
                    eng.dma_start(
                        t[:], src[:, 2 * hp:2 * hp + 2,
                                  blk * 512:(blk + 1) * 512])
                    tiles.append(t)
                return tiles

            def rsqrt_chain(sum_ps, width, tagb):
                """1/sqrt(sum/D + eps) via Ln+Exp (stays in the exp table)."""
                ln_t = small.tile([P, width], F32, name="ln", tag=f"ln{tagb}")
                nc.scalar.activation(ln_t[:], sum_ps[:, 0:width], AF.Ln,
                                     bias=eps_sb[:], scale=1.0 / D)
                rr = small.tile([P, width], F32, name="rr", tag=f"rsq{tagb}")
                nc.scalar.activation(rr[:], ln_t[:], AF.Exp, scale=-0.5)
                return rr

            def make_kv_units(kcol, ct_tiles):
                """K/V projection + k-rmsnorm + V transpose for one key col,
                as 4 fill units (3 matmul batches + finish)."""
                state = {}

                def mms(lo, hi):
                    def f():
                        if lo == 0:
                            state["k"] = psum.tile([P, 512], F32, name="k_ps",
                                                   tag="work", bufs=2)
                            state["v"] = psum.tile([P, 512], F32, name="v_ps",
                                                   tag="work", bufs=2)
                        k_ps, v_ps = state["k"], state["v"]
                        for hc in range(lo, hi):
                            src = ct_tiles[hc // 2][
                                :, (hc % 2) * 512:(hc % 2 + 1) * 512]
                            wv = wkv_sbs[hc // 4]
                            o = (hc % 4) * 256
                            nc.tensor.matmul(
                                k_ps[:], wv[:, o:o + 128],
                                src, start=(hc == 0), stop=(hc == HC - 1))
                            nc.tensor.matmul(
                                v_ps[:], wv[:, o + 128:o + 256],
                                src, start=(hc == 0), stop=(hc == HC - 1))
                    return f

                def finish():
                    k_ps, v_ps = state["k"], state["v"]
                    # copy k/v out of PSUM at once so the work slots free
                    # fast; the rmsnorm chain runs SBUF-side.
                    ktmp = small.tile([P, 512], F16, name="ktmp", tag="ktmp")
                    nc.vector.tensor_copy(ktmp[:], k_ps[:])
                    vT = small.tile([P, 512], F16, name="vT", tag="vT")
                    nc.vector.tensor_copy(vT[:], v_ps[:])
                    # transpose v 128x128 blocks into [keys, D] FIRST: their
                    # PSUM slots recycle via fast DVE copies, keeping the PE
                    # clear of the Scalar-paced rmsnorm chain below.
                    for j in range(4):
                        kt = kcol * 4 + j
                        tp = psum.tile([P, P], F16, name="tp", tag="work",
                                       bufs=2)
                        nc.tensor.transpose(tp[:], vT[:, j * P:(j + 1) * P],
                                            ident_h[:])
                        nc.vector.tensor_copy(v_sb[:, kt * D:(kt + 1) * D],
                                              tp[:])
                    # k rmsnorm over D (partition dim): sumsq via ones matmul
                    ksq = small.tile([P, 512], F16, name="ksq", tag="sq")
                    nc.scalar.square(ksq[:], ktmp[:])
                    ksum = psum.tile([P, 512], F32, name="ksum", tag="work",
                                     bufs=2)
                    nc.tensor.matmul(ksum[:], ones_h[:], ksq[:],
                                     start=True, stop=True)
                    krr = rsqrt_chain(ksum, 512, "k")
                    nc.vector.scalar_tensor_tensor(
                        out=kT_sb[:, kcol * 512:(kcol + 1) * 512],
                        in0=ktmp[:], scalar=nkw_sb[:], in1=krr[:],
                        op0=OP.mult, op1=OP.mult)

                return [("kv", mms(0, 4)), ("kv", mms(4, 10)),
                        ("kv", mms(10, 16)), ("kv", finish)]

            def emit_kv(kcol, ct_tiles):
                for _, fn in make_kv_units(kcol, ct_tiles):
                    fn()

            def qp_head_mms(pb, h, xt_tiles):
                """Q projection matmuls for one head of one 512-query block.

                The PSUM result is copied straight to SBUF (one quick DVE op)
                so the work-psum slot frees immediately; the rmsnorm chain
                then runs SBUF-side without blocking the PE rotation.
                """
                q_ps = psum.tile([P, 512], F32, name=f"q_ps{h}", tag="work",
                                 bufs=2)
                for hc in range(HC):
                    src = xt_tiles[hc // 2][:, (hc % 2) * 512:(hc % 2 + 1) * 512]
                    nc.tensor.matmul(
                        q_ps[:],
                        wq_sb[:, hc * 512 + h * D:hc * 512 + (h + 1) * D],
                        src, start=(hc == 0), stop=(hc == HC - 1))
                qtmp = small.tile([P, 512], F16, name="qtmp",
                                  tag=f"qtmp{h % 2}")
                nc.vector.tensor_copy(qtmp[:], q_ps[:])
                qsq = small.tile([P, 512], F16, name="qsq", tag=f"sq{h % 2}")
                nc.vector.tensor_tensor(out=qsq[:], in0=qtmp[:], in1=qtmp[:],
                                        op=OP.mult)
                return qtmp, qsq

            def qp_head_norm(pb, h, qtmp, qsq):
                """RMSNorm for one head's q block (after mms + square)."""
                qsum = psum.tile([P, 512], F32, name=f"qsum{h}", tag="work",
                                 bufs=2)
                nc.tensor.matmul(qsum[:], ones_h[:], qsq[:],
                                 start=True, stop=True)
                qrr = rsqrt_chain(qsum, 512, f"q{h % 2}")
                nc.vector.scalar_tensor_tensor(
                    out=xqT_list[h][:, pb * 512:(pb + 1) * 512],
                    in0=qtmp[:], scalar=nqw_sb[:], in1=qrr[:],
                    op0=OP.mult, op1=OP.mult)

            def make_qp_fills(pb, xt_tiles):
                """Fill units (kind, closure) for one q-block's projection."""
                units = []
                state = {}

                def mk_a(h):
                    def f():
                        state[h] = qp_head_mms(pb, h, xt_tiles)
                    return f

                def mk_b(h):
                    def f():
                        q_ps, qsq = state.pop(h)
                        qp_head_norm(pb, h, q_ps, qsq)
                    return f

                for h in range(NH):
                    units.append(("qpa", mk_a(h)))
                    units.append(("qpb", mk_b(h)))
                return units

            def make_wo_fills(ab, attn_map):
                """Fill units for one block's output projection (16 chunks)."""
                units = []
                state = {}

                def mk(qs, ht):
                    def f():
                        if ht == 0:
                            state["ot"] = outp.tile([P, 2048], F16, name="ot",
                                                    tag="ot")
                        w_ps = psum.tile([P, 512], F32, name="w_ps",
                                         tag="work", bufs=2)
                        for h in range(NH):
                            nc.tensor.matmul(
                                w_ps[:],
                                attn_map[h][:, qs * P:(qs + 1) * P],
                                wo_sb[:, h * HID + ht * 512:
                                      h * HID + (ht + 1) * 512],
                                start=(h == 0), stop=(h == NH - 1))
                        ot = state["ot"]
                        nc.vector.tensor_copy(
                            ot[:, ht * 512:(ht + 1) * 512], w_ps[:])
                        if ht == 3:
                            q0 = ab * 512 + qs * P
                            nc.sync.dma_start(out[q0:q0 + P, :], ot[:])
                    return f

                ready = lambda: 3 in attn_map  # noqa: E731
                for qs in range(4):
                    for ht in range(4):
                        units.append(("wo", mk(qs, ht), ready))
                return units

            def make_wo_a_fills(ab, attn_map, ota_store):
                """First-pass wo chunks (heads 0-1 only) for the LAST block;
                pumped during its stream once hg0's post has run. Partials
                land in fp16 SBUF for the tail pass to combine."""
                units = []
                ready = lambda: 0 in attn_map  # noqa: E731

                def mk(qs, ht):
                    def f():
                        w_ps = psum.tile([P, 512], F32, name="w_ps",
                                         tag="work", bufs=2)
                        for h in range(2):
                            nc.tensor.matmul(
                                w_ps[:],
                                attn_map[h][:, qs * P:(qs + 1) * P],
                                wo_sb[:, h * HID + ht * 512:
                                      h * HID + (ht + 1) * 512],
                                start=(h == 0), stop=(h == 1))
                        ota = otap.tile([P, 512], F16, name="ota", tag="ota",
                                        bufs=16)
                        nc.vector.tensor_copy(ota[:], w_ps[:])
                        ota_store[(qs, ht)] = ota
                    return f

                for qs in range(4):
                    for ht in range(4):
                        units.append(("wo", mk(qs, ht), ready))
                return units

            def emit_wo_b(ab, attn_map, ota_store):
                """Tail pass: heads 2-3 + combine with the stored partials."""
                for qs in range(4):
                    ot = outp.tile([P, 2048], F16, name="ot", tag="ot")
                    for ht in range(4):
                        w_ps = psum.tile([P, 512], F32, name="w_ps",
                                         tag="work", bufs=2)
                        for h in (2, 3):
                            nc.tensor.matmul(
                                w_ps[:],
                                attn_map[h][:, qs * P:(qs + 1) * P],
                                wo_sb[:, h * HID + ht * 512:
                                      h * HID + (ht + 1) * 512],
                                start=(h == 2), stop=(h == 3))
                        nc.vector.tensor_tensor(
                            out=ot[:, ht * 512:(ht + 1) * 512], in0=w_ps[:],
                            in1=ota_store[(qs, ht)][:], op=OP.add)
                    q0 = ab * 512 + qs * P
                    nc.sync.dma_start(out[q0:q0 + P, :], ot[:])

            class Pump:
                def __init__(self):
                    self.q = []
                    self.cooldown = 0

                def add(self, units):
                    self.q.extend(units)

                def step(self):
                    if self.cooldown > 0:
                        self.cooldown -= 1
                        return
                    if not self.q:
                        return
                    idx = None
                    for i, unit in enumerate(self.q):
                        if len(unit) <= 2 or unit[2]():
                            idx = i
                            break
                    if idx is None:
                        return  # everything gated this tick
                    unit = self.q.pop(idx)
                    kind, fn = unit[0], unit[1]
                    fn()
                    # qpa->qpb needs a few iters for the DVE copy+square to
                    # land before the qsum matmul; wo units are cheap and can
                    # fire every iteration.
                    self.cooldown = {"qpa": 2, "qpb": 0, "wo": 0,
                                     "kv": 0}[kind]

                def drain(self):
                    while self.q:
                        unit = self.q.pop(0)
                        unit[1]()

            pump = Pump()

            def make_post(hg, att0, att1, tree, attn_map):
                """Deferred sump + normalize for one head group; emitted a
                couple of iterations into the NEXT stream so the DVE add
                tree and PSUM frees never stall the PE."""
                def post():
                    acc = tree[3]
                    sump0 = psum.tile([P, 512], F32, name="sump0",
                                      tag="work", bufs=2)
                    nc.tensor.matmul(sump0[:], ones_h[:], acc[:, 0:512],
                                     start=True, stop=True)
                    sump1 = psum.tile([P, 512], F32, name="sump1",
                                      tag="work", bufs=2)
                    nc.tensor.matmul(sump1[:], ones_h[:], acc[:, 512:1024],
                                     start=True, stop=True)
                    for hh, (att, sump) in ((0, (att0, sump0)),
                                            (1, (att1, sump1))):
                        h = 2 * hg + hh
                        rr = small.tile([P, 512], F32, name="rr",
                                        tag=f"den{hh}")
                        nc.vector.reciprocal_approx_fast(out=rr[:],
                                                         in_=sump[:])
                        attn = attnp.tile([P, 512], F16, name=f"attn{h}",
                                          tag="attn")
                        nc.vector.tensor_tensor(
                            out=attn[:], in0=att[:], in1=rr[:], op=OP.mult)
                        attn_map[h] = attn
                return post

            def run_attention(block_fills, attn_maps):
                """All 4 blocks x 2 head groups as ONE software-pipelined
                stream: scores/exp for stream si overlap the P@V tail of
                stream si-1, so the PE and Scalar never re-prime at
                boundaries. Posts (sump+normalize) are deferred into the
                following stream; fill units pump every tick.

                Returns the final deferred post (for the tail).
                """
                NS = 2 * NB
                state = {}
                pending = None
                for t in range(NS * KC + PIPE):
                    if t < NS * KC:
                        si, kt = divmod(t, KC)
                        ab, hg = divmod(si, 2)
                        if kt == 0:
                            if hg == 0:
                                pump.add(block_fills[ab])
                            state[si] = {
                                "es": [None] * KC,
                                "tree": [None] * 5,
                                "att0": psum.tile([P, 512], F32,
                                                  name=f"att{2 * hg}",
                                                  tag="att", bufs=2),
                                "att1": psum.tile([P, 512], F32,
                                                  name=f"att{2 * hg + 1}",
                                                  tag="att", bufs=2),
                            }
                        # the previous stream's trailing AVs are emitted up
                        # to kt==PIPE-1 of this stream; its post must follow
                        # them or the normalize reads a partial accumulator
                        if kt == PIPE and pending is not None:
                            pending()
                            pending = None
                        st = state[si]
                        h0, h1 = 2 * hg, 2 * hg + 1
                        q0 = ab * 512
                        stp = psum.tile([P, 1024], F32, name="stp",
                                        tag="stp", bufs=2)
                        nc.tensor.matmul(stp[:, 0:512],
                                         kT_sb[:, kt * P:(kt + 1) * P],
                                         xqT_list[h0][:, q0:q0 + 512],
                                         start=True, stop=True)
                        nc.tensor.matmul(stp[:, 512:1024],
                                         kT_sb[:, kt * P:(kt + 1) * P],
                                         xqT_list[h1][:, q0:q0 + 512],
                                         start=True, stop=True)
                        es = esbp.tile([P, 1024], F16, name="es", tag="es")
                        nc.scalar.activation(es[:], stp[:], AF.Exp)
                        st["es"][kt] = es
                        # pairwise-tree denominator accumulation on DVE
                        if kt % 2 == 1:
                            tree = st["tree"]
                            t0 = accp.tile([P, 1024], F16, name="acc0",
                                           tag="acc0", bufs=3)
                            nc.vector.tensor_tensor(
                                out=t0[:], in0=st["es"][kt - 1][:],
                                in1=st["es"][kt][:], op=OP.add)
                            lv = 0
                            while tree[lv] is not None:
                                t1 = accp.tile([P, 1024], F16,
                                               name=f"acc{lv + 1}",
                                               tag=f"acc{lv + 1}", bufs=2)
                                nc.vector.tensor_tensor(
                                    out=t1[:], in0=tree[lv][:], in1=t0[:],
                                    op=OP.add)
                                tree[lv] = None
                                t0 = t1
                                lv += 1
                            tree[lv] = t0
                        if kt == KC - 1:
                            pending = make_post(hg, st["att0"], st["att1"],
                                                st["tree"], attn_maps[ab])
                    # P@V leg, lagging PIPE ticks
                    tj = t - PIPE
                    if tj >= 0:
                        sj, ktj = divmod(tj, KC)
                        stj = state[sj]
                        pes = stj["es"][ktj]
                        nc.tensor.matmul(
                            stj["att0"][:], v_sb[:, ktj * D:(ktj + 1) * D],
                            pes[:, 0:512],
                            start=(ktj == 0), stop=(ktj == KC - 1))
                        nc.tensor.matmul(
                            stj["att1"][:], v_sb[:, ktj * D:(ktj + 1) * D],
                            pes[:, 512:1024],
                            start=(ktj == 0), stop=(ktj == KC - 1))
                        if ktj == KC - 1:
                            del state[sj]
                    pump.step()
                return pending

            # ======== schedule ========
            # SP wire:  wkv+ct0 interleaved, ct1, ct2, ct3 (+ out stores).
            # ACT wire: wq, xt0, xt1, xt2, xt3, wo — all land during the
            #           startup phase while the Scalar sequencer is idle.
            # wkv split across both queues; c on SP, x/weights on ACT so the
            # two input streams land in parallel.
            for i in range(4):
                eng = nc.sync if i % 2 == 0 else nc.scalar
                eng.dma_start(wkv_sbs[i][:], wkv[:, 4 * i:4 * (i + 1), :])
            ct0 = []
            for hp in range(8):
                t = cstream.tile([P, 2 * 512], F16, name="st", tag="stream")
                nc.sync.dma_start(t[:], cT[:, 2 * hp:2 * hp + 2, 0:512])
                ct0.append(t)
            xt0 = dma_stream(nc.scalar, xstream, xT, 0)
            nc.scalar.dma_start(wq_sb[:], wq[:, :, :])
            ct1 = dma_stream(nc.sync, cstream, cT, 1)

            # startup: KV cols 0-2 + q-projection of heads 0/1 (block 0's
            # first head group); everything else rides the fill queue.
            qp0 = make_qp_fills(0, xt0)  # [A0 B0 A1 B1 A2 B2 A3 B3]
            emit_kv(0, ct0)
            qp0[0][1]()   # A(h0)
            qp0[2][1]()   # A(h1)
            ct2 = dma_stream(nc.sync, cstream, cT, 2)
            emit_kv(1, ct1)
            xt1 = dma_stream(nc.scalar, xstream, xT, 1)
            qp0[1][1]()   # B(h0)
            qp0[3][1]()   # B(h1)
            ct3 = dma_stream(nc.sync, cstream, cT, 3)
            emit_kv(2, ct2)
            xt2 = dma_stream(nc.scalar, xstream, xT, 2)
            xt3 = dma_stream(nc.scalar, xstream, xT, 3)
            nc.scalar.dma_start(wo_sb[:], wo[:, :, :])

            # one pipelined attention stream over all 4 blocks; KV col 3,
            # QP0's second head group, later q-projections and all output
            # projections ride the fill queue
            attn_maps = [{} for _ in range(NB)]
            ota_store = {}
            wo1_units = make_wo_fills(1, attn_maps[1])
            block_fills = {
                0: (make_kv_units(3, ct3) + qp0[4:]
                    + make_qp_fills(1, xt1) + make_qp_fills(2, xt2)),
                1: make_qp_fills(3, xt3) + make_wo_fills(0, attn_maps[0]),
                2: wo1_units[:12],
                # wo_a first: its guard holds it until block 3's hg0 post;
                # the skip-scan pulls WO1-tail/WO2 units through as fillers,
                # including block 3's hg1 warmup ticks
                3: (wo1_units[12:]
                    + make_wo_a_fills(3, attn_maps[3], ota_store)
                    + make_wo_fills(2, attn_maps[2])),
            }
            pending = run_attention(block_fills, attn_maps)
            # leftover wo_a units cover the DVE add-tree latency before the
            # final post's sump matmuls need it
            pump.drain()
            pending()
            emit_wo_b(3, attn_maps[3], ota_store)

    nc.compile()
    return nc


def _get_compiled():
    global _compiled
    if _compiled is None:
        _compiled = _build()
    return _compiled


def _to3d(a):
    """[rows=HC*P, cols] -> [P, HC, cols] (partition-major chunks)."""
    cols = a.shape[1]
    return np.ascontiguousarray(
        a.reshape(HC, P, cols).transpose(1, 0, 2))


def _shard_inputs(x, c, wq, wkv, wo, norm_q_w, norm_k_w):
    x = np.asarray(x, np.float32)
    c = np.asarray(c, np.float32)
    wq = np.asarray(wq, np.float32)
    wkv = np.asarray(wkv, np.float32)
    wo = np.asarray(wo, np.float32)
    nqw = (np.asarray(norm_q_w, np.float32) * np.float32(SCALE)).reshape(P, 1)
    nkw = np.asarray(norm_k_w, np.float32).reshape(P, 1).copy()

    xTs = [_to3d(np.ascontiguousarray(x[b].T)).astype(np.float16)
           for b in range(B)]
    cTs = [_to3d(np.ascontiguousarray(c[b].T)).astype(np.float16)
           for b in range(B)]
    in_maps = []
    for core in range(8):
        b, g = core // 4, core % 4
        blk = wkv[:, g * 256:(g + 1) * 256]
        kvpack = np.concatenate([blk[:, 0::2], blk[:, 1::2]], axis=1)
        wo_g = wo[g * 512:(g + 1) * 512, :]
        in_maps.append({
            "xT": xTs[b],
            "cT": cTs[b],
            "wq": _to3d(wq[:, g * 512:(g + 1) * 512]).astype(np.float16),
            "wkv": _to3d(kvpack).astype(np.float16),
            "wo": np.ascontiguousarray(
                wo_g.reshape(NH, P, HID).transpose(1, 0, 2)).astype(
                    np.float16),
            "nqw": nqw,
            "nkw": nkw,
        })
    return in_maps


def run_sharded(inputs, trace=False, trace_cores=None):
    """Run the SPMD kernel; returns (full_output, BassKernelResults)."""
    nc = _get_compiled()
    in_maps = _shard_inputs(**inputs)
    res = run_bass_kernel_spmd(nc, in_maps, core_ids=list(range(8)),
                               trace=trace, trace_cores=trace_cores)
    parts = [r["out"] for r in res.results]
    full = np.empty((B, S, HID), np.float32)
    for b in range(B):
        full[b] = np.sum(np.stack([parts[4 * b + g] for g in range(4)], 0),
                         axis=0, dtype=np.float64).astype(np.float32)
    return full, res


def kernel(**inputs) -> np.ndarray:
    out, _ = run_sharded(inputs, trace=False)
    return out
